# revision 14
# baseline (speedup 1.0000x reference)
"""Trainium2 Bass kernel for AttLayer-style attention pooling.

Computes, for x[B, T, D], W[D, A], b[A], u[A, 1]:
    uit = tanh(x @ W + b)            # [B, T, A]
    z   = uit @ u[:, 0]              # [B, T]
    e   = exp(z)
    a   = e / (sum_t e + 1e-7)
    y   = einsum('btd,bt->bd', x, a) # [B, D]

Sharding: pure data parallel over batch. Each of the 8 NeuronCores gets
B/8 = 8 batches; params are replicated; no cross-core communication.

Host-side prep (free relative to device time): x is shipped TRANSPOSED
as xT[b, d, t] in fp16, so the kernel needs no on-device transposes at
all; u is shipped replicated to 128 columns so mm2 produces z already
broadcast across all partitions.

Per-core, per-batch dataflow:
  1. One DMA loads xT into SBUF as [128, 2, T] fp16 (partition d holds
     d-chunks c=0/1; 4 KiB contiguous reads per (d, c)).
  2. mm1: W-chunk-stationary matmuls accumulate uitT = W^T xT in a
     4-bank PSUM tile [A, 2048]; one ScalarE tanh(+b) writes uitT to
     SBUF as fp16.
  3. mm2: stationary U128 = u*ones[1,128] gives z_rep[p, t] = z[t] for
     every partition p, in two 2-bank PSUM halves; ScalarE exp writes
     e[128, 2048] fp16 with accum_out giving S = sum_t e[t] on every
     partition. The h1 exp is software-pipelined one batch behind so
     the ScalarE never stalls on mm2.
  4. VectorE: r = 1/(S+eps); two fused scalar_tensor_tensor ops compute
     y[d] = sum_t (xT[d, t] * r) * e[t] per d-chunk -- normalization is
     folded into the pooling pass, accum lands directly in y_all.
  5. A final PE transpose folds y[128, BC, 2] into [2*BC, 128] so one
     16-descriptor DMA writes the full [BC, D] output.
"""

from contextlib import ExitStack

import numpy as np

import concourse.bass as bass
import concourse.tile as tile
from concourse import mybir
from concourse.bass_utils import run_bass_kernel_spmd
from concourse.masks import make_identity

N_CORES = 8
B, T, D, A = 64, 2048, 256, 128
BC = B // N_CORES  # batches per core
TH = T // 2  # exp half size
TC = 512  # matmul free-dim chunk (one PSUM bank)
EPS = 1e-7

F32 = mybir.dt.float32
F16 = mybir.dt.float16
TANH = mybir.ActivationFunctionType.Tanh
EXP = mybir.ActivationFunctionType.Exp
MULT = mybir.AluOpType.mult
ADD = mybir.AluOpType.add


def _split_multi_waits(nc):
    """Hoist all-but-one sem wait off restricted instructions onto no-ops.

    The walrus build in this container rejects instructions carrying more
    than one sync-wait command (CoreV3 setupSyncWait). A no-op on the same
    engine immediately before the instruction is semantically identical:
    the engine blocks on each wait in sequence.
    """
    counter = [0]

    def fresh_nop(engine, wait):
        counter[0] += 1
        n = mybir.InstNoOp(name=f"I-waitsplit-{counter[0]}", ins=[], outs=[])
        n.engine = engine
        n.sync_info = mybir.SyncInfo(on_wait=[wait], on_update=[])
        nc.register_instruction(n)
        return n

    for fn in nc.m.functions:
        for blk in fn.blocks:
            changed = False
            out = []
            for inst in blk.instructions:
                si = inst.sync_info
                if si is not None and si.on_wait and len(si.on_wait) > 1:
                    waits = list(si.on_wait)
                    for w in waits[:-1]:
                        out.append(fresh_nop(inst.engine, w))
                    si.on_wait = waits[-1:]
                    changed = True
                out.append(inst)
            if changed:
                blk.instructions = out


def _emit_body(ctx, tc, xt, wc, u128, bb, out, repeat=1, hw_loop=False):
    nc = tc.nc

    singles = ctx.enter_context(tc.tile_pool(name="singles", bufs=1))
    xpool = ctx.enter_context(tc.tile_pool(name="xt", bufs=3))
    upool = ctx.enter_context(tc.tile_pool(name="uit", bufs=2))
    epool = ctx.enter_context(tc.tile_pool(name="e", bufs=2))
    spool = ctx.enter_context(tc.tile_pool(name="small", bufs=4))
    scrpool = ctx.enter_context(tc.tile_pool(name="scr", bufs=2))
    pu_pool = ctx.enter_context(tc.tile_pool(name="pu", bufs=1, space="PSUM"))
    pz0_pool = ctx.enter_context(tc.tile_pool(name="pz0", bufs=1, space="PSUM"))
    pz1_pool = ctx.enter_context(tc.tile_pool(name="pz1", bufs=1, space="PSUM"))

    # Replicated parameters, shipped pre-cast/pre-laid-out from the host.
    wc_sb = singles.tile([128, 2, A], F16)
    nc.sync.dma_start(wc_sb[:], wc.ap())
    u_sb = singles.tile([A, 128], F16)
    nc.sync.dma_start(u_sb[:], u128.ap())
    b_sb = singles.tile([A, 1], F32)
    nc.sync.dma_start(b_sb[:], bb.ap())
    idf = singles.tile([128, 128], F32)
    make_identity(nc, idf[:])
    y_all = singles.tile([128, BC, 2], F32)

    def load_pair(pr, split_first=False):
        # One 2 MiB DMA per batch pair: each partition reads a single
        # 16 KiB contiguous run (the host ships [pair, d, b2, c, t]).
        # The first pair is split in two so mm1(b0) starts ~2.5us sooner
        # (shorter pipeline fill on a one-shot pass).
        xt_sb = xpool.tile([128, 2, 2, T], F16, tag="xt")
        if split_first:
            nc.sync.dma_start(xt_sb[:, 0], xt.ap()[pr, :, 0])
            nc.sync.dma_start(xt_sb[:, 1], xt.ap()[pr, :, 1])
        else:
            nc.sync.dma_start(xt_sb[:], xt.ap()[pr])
        return xt_sb

    def mm1(xt_sb, j):
        # uitT[a, t] = tanh(sum_d W[d, a] x[t, d] + b[a]); W0 pass then W1
        # pass so the stationary swaps twice per batch instead of eight.
        pu = pu_pool.tile([A, T], F32, tag="pu")
        for kc in range(2):
            for tcn in range(T // TC):
                sl = slice(TC * tcn, TC * (tcn + 1))
                nc.tensor.matmul(
                    pu[:, sl],
                    wc_sb[:, kc, :],
                    xt_sb[:, j, kc, sl],
                    start=(kc == 0),
                    stop=(kc == 1),
                )
        return pu

    def tanh(pu):
        uitT = upool.tile([A, T], F16, tag="uitT")
        nc.scalar.activation(uitT[:], pu[:], TANH, bias=b_sb[:])
        return uitT

    def mm2_half(uitT, h):
        pool = pz0_pool if h == 0 else pz1_pool
        pz = pool.tile([128, TH], F32, tag=f"pz{h}")
        for tcn in range(TH // TC):
            sl = slice(TC * tcn, TC * (tcn + 1))
            nc.tensor.matmul(
                pz[:, sl], u_sb[:], uitT[:, h * TH + TC * tcn :][:, : TC],
                start=True, stop=True,
            )
        return pz

    def exp_half(st, h):
        e_sb = st["e"]
        s = spool.tile([128, 1], F32, tag=f"s{h}", name=f"s{h}")
        nc.scalar.activation(
            e_sb[:, h * TH : (h + 1) * TH], st[f"pz{h}"][:], EXP, accum_out=s[:]
        )
        st[f"s{h}"] = s

    def consume(st, bi):
        # S = s0 + s1; r = 1/(S+eps); y[d] = sum_t (xT[d,t]*r)*e[t]
        ssum = spool.tile([128, 1], F32, tag="ssum")
        nc.vector.tensor_tensor(ssum[:], st["s0"][:], st["s1"][:], ADD)
        sc = spool.tile([128, 1], F32, tag="sc")
        nc.vector.tensor_scalar_add(sc[:], ssum[:], EPS)
        r1 = spool.tile([128, 1], F32, tag="r1")
        nc.vector.reciprocal(r1[:], sc[:])
        for c in (0, 1):
            scratch = scrpool.tile([128, T], F16, tag=f"scr{c}")
            nc.vector.scalar_tensor_tensor(
                out=scratch[:],
                in0=st["xt"][:, st["j"], c, :],
                scalar=r1[:],
                in1=st["e"][:],
                op0=MULT,
                op1=MULT,
                accum_out=y_all[:, bi, c : c + 1],
            )

    def one_pass():
        pairs = {}

        def ensure_pair(pr, split_first=False):
            if pr < BC // 2 and pr not in pairs:
                pairs[pr] = load_pair(pr, split_first)

        ensure_pair(0, split_first=True)
        ensure_pair(1)
        sts = {bi: {"xt": pairs[bi // 2], "j": bi % 2} for bi in range(2)}
        pu = mm1(sts[0]["xt"], 0)
        for bi in range(BC):
            st = sts[bi]
            st["uitT"] = tanh(pu)
            if bi > 0:
                exp_half(sts[bi - 1], 1)
                consume(sts[bi - 1], bi - 1)
                del sts[bi - 1]
            st["e"] = epool.tile([128, T], F16, tag="e", name="e_sb")
            st["pz0"] = mm2_half(st["uitT"], 0)
            exp_half(st, 0)
            if bi + 1 < BC:
                if bi + 2 < BC:
                    ensure_pair((bi + 2) // 2)
                    sts[bi + 2] = {"xt": pairs[(bi + 2) // 2], "j": (bi + 2) % 2}
                pu = mm1(sts[bi + 1]["xt"], sts[bi + 1]["j"])
            st["pz1"] = mm2_half(st["uitT"], 1)
        exp_half(sts[BC - 1], 1)
        consume(sts[BC - 1], BC - 1)

        # Fold y_all[d, (b, c)] -> [(b, c), d] so the output DMA is 16
        # contiguous 512B rows.
        ytp = pz1_pool.tile([128, TH], F32, tag="pz1")
        nc.tensor.transpose(
            ytp[0 : 2 * BC, 0:128], y_all[:].rearrange("d b c -> d (b c)"), idf[:]
        )
        yts = spool.tile([2 * BC, 128], F32, tag="yts")
        nc.vector.tensor_copy(yts[:], ytp[0 : 2 * BC, 0:128])
        nc.sync.dma_start(out.ap().rearrange("b (c d) -> (b c) d", c=2), yts[:])

    if hw_loop and repeat > 1:
        with tc.For_i(0, repeat, 1):
            one_pass()
    else:
        for _ in range(repeat):
            one_pass()


_NC_CACHE = {}


def _build_nc(repeat=1, hw_loop=False):
    key = (repeat, hw_loop)
    if key in _NC_CACHE:
        return _NC_CACHE[key]
    nc = bass.Bass()
    xt = nc.declare_dram_parameter("xt", [BC // 2, 128, 2, 2, T], F16, isOutput=False)
    wc = nc.declare_dram_parameter("wc", [128, 2, A], F16, isOutput=False)
    u128 = nc.declare_dram_parameter("u128", [A, 128], F16, isOutput=False)
    bb = nc.declare_dram_parameter("bb", [A, 1], F32, isOutput=False)
    out = nc.declare_dram_parameter("out", [BC, D], F32, isOutput=True)
    with tile.TileContext(nc) as tc, ExitStack() as ctx:
        _emit_body(ctx, tc, xt, wc, u128, bb, out, repeat=repeat, hw_loop=hw_loop)
    _split_multi_waits(nc)
    _NC_CACHE[key] = nc
    return nc


def make_in_maps(x, W, b, u):
    x = np.asarray(x, dtype=np.float32)
    # [B, T, D] f32 -> fp16, then lay out as [pair, d, b2, c, t] so each
    # SBUF partition reads one 16 KiB contiguous run per 2 MiB pair-DMA.
    # Built with one strided copy: both sides are pure views.
    x16 = x.astype(np.float16)
    xt = np.empty((B // 2, 128, 2, 2, T), dtype=np.float16)
    np.copyto(
        xt.transpose(0, 2, 3, 1, 4),
        x16.reshape(B // 2, 2, T, 2, 128).transpose(0, 1, 3, 4, 2),
    )
    wc = np.ascontiguousarray(
        np.asarray(W, dtype=np.float32).reshape(2, 128, A).transpose(1, 0, 2)
    ).astype(np.float16)
    u128 = np.ascontiguousarray(
        np.broadcast_to(np.asarray(u, dtype=np.float32).reshape(A, 1), (A, 128))
    ).astype(np.float16)
    bb = np.asarray(b, dtype=np.float32).reshape(A, 1).copy()
    return [
        {"xt": xt[c * (BC // 2) : (c + 1) * (BC // 2)], "wc": wc, "u128": u128, "bb": bb}
        for c in range(N_CORES)
    ]


def kernel(x, W, b, u):
    nc = _build_nc()
    res = run_bass_kernel_spmd(nc, make_in_maps(x, W, b, u), list(range(N_CORES)))
    return np.concatenate([r["out"] for r in res.results], axis=0)


# revision 15
# speedup vs baseline: 4.1656x; 4.1656x over previous
"""Trainium2 Bass kernel for AttLayer-style attention pooling.

Computes, for x[B, T, D], W[D, A], b[A], u[A, 1]:
    uit = tanh(x @ W + b)            # [B, T, A]
    z   = uit @ u[:, 0]              # [B, T]
    e   = exp(z)
    a   = e / (sum_t e + 1e-7)
    y   = einsum('btd,bt->bd', x, a) # [B, D]

Sharding: pure data parallel over batch. Each of the 8 NeuronCores gets
B/8 = 8 batches; params are replicated; no cross-core communication.

Host-side prep (free relative to device time): x is shipped TRANSPOSED
as xT[b, d, t] in fp16, so the kernel needs no on-device transposes at
all; u is shipped replicated to 128 columns so mm2 produces z already
broadcast across all partitions.

Per-core, per-batch dataflow:
  1. One DMA loads xT into SBUF as [128, 2, T] fp16 (partition d holds
     d-chunks c=0/1; 4 KiB contiguous reads per (d, c)).
  2. mm1: W-chunk-stationary matmuls accumulate uitT = W^T xT in a
     4-bank PSUM tile [A, 2048]; one ScalarE tanh(+b) writes uitT to
     SBUF as fp16.
  3. mm2: stationary U128 = u*ones[1,128] gives z_rep[p, t] = z[t] for
     every partition p, in two 2-bank PSUM halves; ScalarE exp writes
     e[128, 2048] fp16 with accum_out giving S = sum_t e[t] on every
     partition. The h1 exp is software-pipelined one batch behind so
     the ScalarE never stalls on mm2.
  4. VectorE: r = 1/(S+eps); two fused scalar_tensor_tensor ops compute
     y[d] = sum_t (xT[d, t] * r) * e[t] per d-chunk -- normalization is
     folded into the pooling pass, accum lands directly in y_all.
  5. A final PE transpose folds y[128, BC, 2] into [2*BC, 128] so one
     16-descriptor DMA writes the full [BC, D] output.
"""

from contextlib import ExitStack

import numpy as np

import concourse.bass as bass
import concourse.tile as tile
from concourse import mybir
from concourse.bass_utils import run_bass_kernel_spmd
from concourse.masks import make_identity

N_CORES = 8
B, T, D, A = 64, 2048, 256, 128
BC = B // N_CORES  # batches per core
TH = T // 2  # exp half size
TC = 512  # matmul free-dim chunk (one PSUM bank)
EPS = 1e-7

F32 = mybir.dt.float32
F16 = mybir.dt.float16
TANH = mybir.ActivationFunctionType.Tanh
EXP = mybir.ActivationFunctionType.Exp
MULT = mybir.AluOpType.mult
ADD = mybir.AluOpType.add


def _split_multi_waits(nc):
    """Hoist all-but-one sem wait off restricted instructions onto no-ops.

    The walrus build in this container rejects instructions carrying more
    than one sync-wait command (CoreV3 setupSyncWait). A no-op on the same
    engine immediately before the instruction is semantically identical:
    the engine blocks on each wait in sequence.
    """
    counter = [0]

    def fresh_nop(engine, wait):
        counter[0] += 1
        n = mybir.InstNoOp(name=f"I-waitsplit-{counter[0]}", ins=[], outs=[])
        n.engine = engine
        n.sync_info = mybir.SyncInfo(on_wait=[wait], on_update=[])
        nc.register_instruction(n)
        return n

    for fn in nc.m.functions:
        for blk in fn.blocks:
            changed = False
            out = []
            for inst in blk.instructions:
                si = inst.sync_info
                if si is not None and si.on_wait and len(si.on_wait) > 1:
                    waits = list(si.on_wait)
                    for w in waits[:-1]:
                        out.append(fresh_nop(inst.engine, w))
                    si.on_wait = waits[-1:]
                    changed = True
                out.append(inst)
            if changed:
                blk.instructions = out


def _emit_body(ctx, tc, xt, wc, u128, bb, out, repeat=1, hw_loop=False):
    nc = tc.nc

    singles = ctx.enter_context(tc.tile_pool(name="singles", bufs=1))
    xpool = ctx.enter_context(tc.tile_pool(name="xt", bufs=2))
    upool = ctx.enter_context(tc.tile_pool(name="uit", bufs=2))
    epool = ctx.enter_context(tc.tile_pool(name="e", bufs=2))
    spool = ctx.enter_context(tc.tile_pool(name="small", bufs=4))
    scrpool = ctx.enter_context(tc.tile_pool(name="scr", bufs=2))
    pu_pool = ctx.enter_context(tc.tile_pool(name="pu", bufs=1, space="PSUM"))
    pz0_pool = ctx.enter_context(tc.tile_pool(name="pz0", bufs=1, space="PSUM"))
    pz1_pool = ctx.enter_context(tc.tile_pool(name="pz1", bufs=1, space="PSUM"))

    # Replicated parameters, shipped pre-cast/pre-laid-out from the host.
    wc_sb = singles.tile([128, 2, A], F16)
    nc.sync.dma_start(wc_sb[:], wc.ap())
    u_sb = singles.tile([A, 128], F16)
    nc.sync.dma_start(u_sb[:], u128.ap())
    b_sb = singles.tile([A, 1], F32)
    nc.sync.dma_start(b_sb[:], bb.ap())
    idf = singles.tile([128, 128], F32)
    make_identity(nc, idf[:])
    y_all = singles.tile([128, BC, 2], F32)

    def load_quad(qr, split_first=False):
        # One 4 MiB DMA per 4-batch group: each partition reads a single
        # 32 KiB contiguous run (the host ships [quad, d, b4, c, t]).
        xt_sb = xpool.tile([128, 4, 2, T], F16, tag="xt")
        if split_first:
            nc.sync.dma_start(xt_sb[:, 0], xt.ap()[qr, :, 0])
            nc.sync.dma_start(xt_sb[:, 1:], xt.ap()[qr, :, 1:])
        else:
            nc.sync.dma_start(xt_sb[:], xt.ap()[qr])
        return xt_sb

    def mm1(xt_sb, j):
        # uitT[a, t] = tanh(sum_d W[d, a] x[t, d] + b[a]); W0 pass then W1
        # pass so the stationary swaps twice per batch instead of eight.
        pu = pu_pool.tile([A, T], F32, tag="pu")
        for kc in range(2):
            for tcn in range(T // TC):
                sl = slice(TC * tcn, TC * (tcn + 1))
                nc.tensor.matmul(
                    pu[:, sl],
                    wc_sb[:, kc, :],
                    xt_sb[:, j, kc, sl],
                    start=(kc == 0),
                    stop=(kc == 1),
                )
        return pu

    def tanh(pu):
        uitT = upool.tile([A, T], F16, tag="uitT")
        nc.scalar.activation(uitT[:], pu[:], TANH, bias=b_sb[:])
        return uitT

    def mm2_half(uitT, h):
        pool = pz0_pool if h == 0 else pz1_pool
        pz = pool.tile([128, TH], F32, tag=f"pz{h}")
        for tcn in range(TH // TC):
            sl = slice(TC * tcn, TC * (tcn + 1))
            nc.tensor.matmul(
                pz[:, sl], u_sb[:], uitT[:, h * TH + TC * tcn :][:, : TC],
                start=True, stop=True,
            )
        return pz

    def exp_half(st, h):
        e_sb = st["e"]
        s = spool.tile([128, 1], F32, tag=f"s{h}", name=f"s{h}")
        nc.scalar.activation(
            e_sb[:, h * TH : (h + 1) * TH], st[f"pz{h}"][:], EXP, accum_out=s[:]
        )
        st[f"s{h}"] = s

    def consume(st, bi):
        # S = s0 + s1; r = 1/(S+eps); y[d] = sum_t (xT[d,t]*r)*e[t]
        ssum = spool.tile([128, 1], F32, tag="ssum")
        nc.vector.tensor_tensor(ssum[:], st["s0"][:], st["s1"][:], ADD)
        sc = spool.tile([128, 1], F32, tag="sc")
        nc.vector.tensor_scalar_add(sc[:], ssum[:], EPS)
        r1 = spool.tile([128, 1], F32, tag="r1")
        nc.vector.reciprocal(r1[:], sc[:])
        for c in (0, 1):
            scratch = scrpool.tile([128, T], F16, tag=f"scr{c}")
            nc.vector.scalar_tensor_tensor(
                out=scratch[:],
                in0=st["xt"][:, st["j"], c, :],
                scalar=r1[:],
                in1=st["e"][:],
                op0=MULT,
                op1=MULT,
                accum_out=y_all[:, bi, c : c + 1],
            )

    def one_pass():
        quads = {}

        def ensure_quad(qr, split_first=False):
            if qr < BC // 4 and qr not in quads:
                quads[qr] = load_quad(qr, split_first)

        ensure_quad(0, split_first=True)
        sts = {bi: {"xt": quads[bi // 4], "j": bi % 4} for bi in range(2)}
        pu = mm1(sts[0]["xt"], 0)
        for bi in range(BC):
            st = sts[bi]
            st["uitT"] = tanh(pu)
            if bi > 0:
                exp_half(sts[bi - 1], 1)
                consume(sts[bi - 1], bi - 1)
                del sts[bi - 1]
            st["e"] = epool.tile([128, T], F16, tag="e", name="e_sb")
            st["pz0"] = mm2_half(st["uitT"], 0)
            exp_half(st, 0)
            if bi + 1 < BC:
                # Issue the next quad's 4 MiB DMA a full group ahead: it
                # takes ~10.5 us, i.e. ~3.5 batch slots.
                ensure_quad((bi + 4) // 4)
                if bi + 2 < BC:
                    sts[bi + 2] = {"xt": quads[(bi + 2) // 4], "j": (bi + 2) % 4}
                pu = mm1(sts[bi + 1]["xt"], sts[bi + 1]["j"])
            st["pz1"] = mm2_half(st["uitT"], 1)
        exp_half(sts[BC - 1], 1)
        consume(sts[BC - 1], BC - 1)

        # Fold y_all[d, (b, c)] -> [(b, c), d] so the output DMA is 16
        # contiguous 512B rows.
        ytp = pz1_pool.tile([128, TH], F32, tag="pz1")
        nc.tensor.transpose(
            ytp[0 : 2 * BC, 0:128], y_all[:].rearrange("d b c -> d (b c)"), idf[:]
        )
        yts = spool.tile([2 * BC, 128], F32, tag="yts")
        nc.vector.tensor_copy(yts[:], ytp[0 : 2 * BC, 0:128])
        nc.sync.dma_start(out.ap().rearrange("b (c d) -> (b c) d", c=2), yts[:])

    if hw_loop and repeat > 1:
        with tc.For_i(0, repeat, 1):
            one_pass()
    else:
        for _ in range(repeat):
            one_pass()


_NC_CACHE = {}


def _build_nc(repeat=1, hw_loop=False):
    key = (repeat, hw_loop)
    if key in _NC_CACHE:
        return _NC_CACHE[key]
    nc = bass.Bass()
    xt = nc.declare_dram_parameter("xt", [BC // 4, 128, 4, 2, T], F16, isOutput=False)
    wc = nc.declare_dram_parameter("wc", [128, 2, A], F16, isOutput=False)
    u128 = nc.declare_dram_parameter("u128", [A, 128], F16, isOutput=False)
    bb = nc.declare_dram_parameter("bb", [A, 1], F32, isOutput=False)
    out = nc.declare_dram_parameter("out", [BC, D], F32, isOutput=True)
    with tile.TileContext(nc) as tc, ExitStack() as ctx:
        _emit_body(ctx, tc, xt, wc, u128, bb, out, repeat=repeat, hw_loop=hw_loop)
    _split_multi_waits(nc)
    _NC_CACHE[key] = nc
    return nc


def make_in_maps(x, W, b, u):
    x = np.asarray(x, dtype=np.float32)
    # [B, T, D] f32 -> fp16, then lay out as [pair, d, b2, c, t] so each
    # SBUF partition reads one 16 KiB contiguous run per 2 MiB pair-DMA.
    # Built with one strided copy: both sides are pure views.
    x16 = x.astype(np.float16)
    xt = np.empty((B // 4, 128, 4, 2, T), dtype=np.float16)
    np.copyto(
        xt.transpose(0, 2, 3, 1, 4),
        x16.reshape(B // 4, 4, T, 2, 128).transpose(0, 1, 3, 4, 2),
    )
    wc = np.ascontiguousarray(
        np.asarray(W, dtype=np.float32).reshape(2, 128, A).transpose(1, 0, 2)
    ).astype(np.float16)
    u128 = np.ascontiguousarray(
        np.broadcast_to(np.asarray(u, dtype=np.float32).reshape(A, 1), (A, 128))
    ).astype(np.float16)
    bb = np.asarray(b, dtype=np.float32).reshape(A, 1).copy()
    return [
        {"xt": xt[c * (BC // 4) : (c + 1) * (BC // 4)], "wc": wc, "u128": u128, "bb": bb}
        for c in range(N_CORES)
    ]


def kernel(x, W, b, u):
    nc = _build_nc()
    res = run_bass_kernel_spmd(nc, make_in_maps(x, W, b, u), list(range(N_CORES)))
    return np.concatenate([r["out"] for r in res.results], axis=0)
